# revision 3
# baseline (speedup 1.0000x reference)
"""GCN encoder (2-layer VGAE-style) on 8 Trainium2 NeuronCores.

Strategy (graph/data parallel, per sharding hint):
- Destination nodes are partitioned across the 8 cores (6250 each); the small
  weight matrices are replicated.
- Each core aggregates messages for its own destination nodes.  Message
  gathers use dma_gather (int16 indices -> the feature table is addressed in
  two slices split at device-row 32000).
- Host-side graph preprocessing ("METIS-like" partitioning per the hint):
  per-core nodes are re-packed into 50 blocks of <=128 nodes balancing the
  per-block edge counts so every core runs the identical SPMD program;
  normalization constants (deg/dinv, standard cached gcn_norm metadata) are
  computed from edge_index on the host.
- Layer-1 table x1 = dinv * (x @ W1) is built shard-wise on device and
  AllGathered (bf16).  Layer-2 table t2 = dinv * (h @ [W_mu|W_ls]) likewise
  (f32).  Self-loop terms enter via an identity matmul; biases via a rank-1
  (sqrt(deg) x b) PSUM-init matmul, so the whole normalization happens on
  device.
"""

import os
import sys

sys.path.insert(0, "/opt/trn_rl_repo")

import numpy as np
import ml_dtypes

import concourse.bass as bass
import concourse.bacc as bacc
import concourse.mybir as mybir
import concourse.tile as tile
from concourse.bass import AP
from concourse.bass_utils import run_bass_kernel_spmd

# ----------------------------------------------------------------------------
N = 50000
NC = 8
NBPC = N // NC            # 6250 nodes per core
NBLK = 50                 # psum blocks per core
ROWS_PER_CORE = NBLK * 128    # 6400 device rows per core
DEV_ROWS = NC * ROWS_PER_CORE  # 51200
SPLIT = 5 * ROWS_PER_CORE      # 32000: table A = dev rows [0, 32000)
D1 = 128                  # input / hidden feature dim
D2 = 64                   # concat(mu, logstd) output dim
CHUNKS_PER_GATHER = 50    # 6400 indices per dma_gather instruction

F32 = mybir.dt.float32
BF16 = mybir.dt.bfloat16
I16 = mybir.dt.int16

LAST_RESULTS = None       # test harness reads profiling info from here


# ----------------------------------------------------------------------------
# Host-side graph preprocessing
# ----------------------------------------------------------------------------

def _pack_core(nodes, degA, degB):
    """Pack `nodes` into NBLK blocks of <=128 nodes, balancing A/B edge loads.
    Returns (blocks: list[list[node]], maxA, maxB)."""
    order = np.argsort(-(degA + degB), kind="stable")
    loadA = np.zeros(NBLK, np.int64)
    loadB = np.zeros(NBLK, np.int64)
    cnt = np.zeros(NBLK, np.int64)
    blocks = [[] for _ in range(NBLK)]
    wA = 1.0 / max(1.0, degA.sum() / NBLK)   # normalize per-stream loads
    wB = 1.0 / max(1.0, degB.sum() / NBLK)
    for idx in order:
        da, db = degA[idx], degB[idx]
        # feasible blocks (node count)
        score = np.maximum((loadA + da) * wA, (loadB + db) * wB)
        score[cnt >= 128] = np.inf
        b = int(np.argmin(score))
        blocks[b].append(nodes[idx])
        loadA[b] += da
        loadB[b] += db
        cnt[b] += 1
    return blocks, int(loadA.max()), int(loadB.max())


def _preprocess(edge_index, y_edge_index):
    ei = np.concatenate([np.asarray(edge_index), np.asarray(y_edge_index)], axis=1)
    src = ei[0].astype(np.int64)
    dst = ei[1].astype(np.int64)

    deg = np.bincount(dst, minlength=N).astype(np.float64) + 1.0
    dinv = (1.0 / np.sqrt(deg)).astype(np.float32)
    sqd = np.sqrt(deg).astype(np.float32)

    is_a_src = src < SPLIT // ROWS_PER_CORE * NBPC  # src node in cores 0-4

    # degrees per node split by src-table half (excluding self loops, which we
    # handle via the identity matmul; existing (i,i) edges stay in the list)
    degA_all = np.bincount(dst[is_a_src], minlength=N)
    degB_all = np.bincount(dst[~is_a_src], minlength=N)

    core_blocks = []
    maxA = maxB = 0
    for c in range(NC):
        lo, hi = c * NBPC, (c + 1) * NBPC
        nodes = np.arange(lo, hi)
        blocks, mA, mB = _pack_core(nodes, degA_all[lo:hi], degB_all[lo:hi])
        core_blocks.append(blocks)
        maxA = max(maxA, mA)
        maxB = max(maxB, mB)

    kA = max(1, -(-maxA // 128))
    kB = max(1, -(-maxB // 128))

    # device row assignment
    node_devrow = np.empty(N, np.int64)
    devrow_node = np.full(DEV_ROWS, -1, np.int64)
    for c in range(NC):
        for b, blk in enumerate(core_blocks[c]):
            base = c * ROWS_PER_CORE + b * 128
            for p, n in enumerate(blk):
                node_devrow[n] = base + p
                devrow_node[base + p] = n
    assert (node_devrow >= 0).all()
    # consistency: src < 31250  <=>  devrow < SPLIT
    assert ((node_devrow < SPLIT) == (np.arange(N) < 5 * NBPC)).all()

    src_dev = node_devrow[src]
    dst_dev = node_devrow[dst]

    nIA = kA  # gather instructions per stream (50 chunks each)
    nIB = kB

    per_core = []
    for c in range(NC):
        m = (dst >= c * NBPC) & (dst < (c + 1) * NBPC)
        es, ed = src_dev[m], dst_dev[m] - c * ROWS_PER_CORE
        blk = ed // 128
        dloc = ed % 128
        isa = es < SPLIT

        # slot-major edge layout: for each block, A-edges then B-edges
        idxA = np.zeros((nIA, CHUNKS_PER_GATHER, 128), np.int16)
        idxB = np.zeros((nIB, CHUNKS_PER_GATHER, 128), np.int16)
        nslot = NBLK * (kA + kB)
        dstloc = np.full((128, nslot), -1.0, np.float32)

        for b in range(NBLK):
            for is_a in (True, False):
                sel = (blk == b) & (isa == is_a)
                e_src = es[sel] if is_a else es[sel] - SPLIT
                e_dl = dloc[sel]
                k = kA if is_a else kB
                assert len(e_src) <= k * 128, (c, b, is_a, len(e_src))
                for t in range(len(e_src)):
                    j, lane = t // 128, t % 128
                    slot_g = b * k + j          # slot in the A/B stream
                    gi, pos = slot_g // CHUNKS_PER_GATHER, slot_g % CHUNKS_PER_GATHER
                    if is_a:
                        idxA[gi, pos, lane] = e_src[t]
                    else:
                        idxB[gi, pos, lane] = e_src[t]
                    jj = b * (kA + kB) + (j if is_a else kA + j)
                    dstloc[lane, jj] = e_dl[t]

        # wire layout: [nI, 128, 400] where [g, i%16 + 16r, i//16] = idx_i
        def wire(a):
            nI = a.shape[0]
            flat = a.reshape(nI, CHUNKS_PER_GATHER * 128)
            w = flat.reshape(nI, -1, 16).transpose(0, 2, 1)  # [nI, 16, n/16]
            return np.tile(w, (1, 8, 1)).astype(np.int16)

        rows = devrow_node[c * ROWS_PER_CORE:(c + 1) * ROWS_PER_CORE]
        valid = rows >= 0
        dinv_sb = np.zeros((128, NBLK), np.float32)
        sqd_row = np.zeros((1, ROWS_PER_CORE), np.float32)
        dv = np.zeros(ROWS_PER_CORE, np.float32)
        dv[valid] = dinv[rows[valid]]
        dinv_sb[:, :] = dv.reshape(NBLK, 128).T
        sq = np.zeros(ROWS_PER_CORE, np.float32)
        sq[valid] = sqd[rows[valid]]
        sqd_row[0] = sq

        per_core.append(dict(
            idxA=wire(idxA), idxB=wire(idxB), dstloc=dstloc,
            dinv_sb=dinv_sb, sqd_row=sqd_row, rows=rows,
        ))

    return per_core, kA, kB, devrow_node, node_devrow


# ----------------------------------------------------------------------------
# Device program
# ----------------------------------------------------------------------------

def _build_program(kA, kB):
    nIA, nIB = kA, kB
    CC = kA + kB
    NSLOT = NBLK * CC
    NWIRE = CHUNKS_PER_GATHER * 128 // 16  # 400

    nc = bacc.Bacc("TRN2", target_bir_lowering=False, debug=False,
                   num_devices=NC)

    # inputs
    xT = nc.dram_tensor("xT", [128, ROWS_PER_CORE], F32, kind="ExternalInput")
    W1 = nc.dram_tensor("W1", [D1, D1], F32, kind="ExternalInput")
    W1b = nc.dram_tensor("W1b", [D1, D1], BF16, kind="ExternalInput")
    W2b = nc.dram_tensor("W2b", [D1, D2], BF16, kind="ExternalInput")
    b1r = nc.dram_tensor("b1r", [1, D1], F32, kind="ExternalInput")
    b2r = nc.dram_tensor("b2r", [1, D2], F32, kind="ExternalInput")
    iota = nc.dram_tensor("iota", [128, 128], F32, kind="ExternalInput")
    identb = nc.dram_tensor("identb", [128, 128], BF16, kind="ExternalInput")
    identf = nc.dram_tensor("identf", [128, 128], F32, kind="ExternalInput")
    dstloc_d = nc.dram_tensor("dstloc", [128, NSLOT], F32, kind="ExternalInput")
    dinv_d = nc.dram_tensor("dinv_sb", [128, NBLK], F32, kind="ExternalInput")
    sqd_d = nc.dram_tensor("sqd_row", [1, ROWS_PER_CORE], F32, kind="ExternalInput")
    idxA_d = nc.dram_tensor("idxA", [nIA, 128, NWIRE], I16, kind="ExternalInput")
    idxB_d = nc.dram_tensor("idxB", [nIB, 128, NWIRE], I16, kind="ExternalInput")

    zcat = nc.dram_tensor("zcat", [ROWS_PER_CORE, D2], F32, kind="ExternalOutput")

    # internal DRAM
    x1_part = nc.dram_tensor("x1_part", [ROWS_PER_CORE, D1], BF16)
    x1_full = nc.dram_tensor("x1_full", [DEV_ROWS, D1], BF16, addr_space="Shared")
    t2_part = nc.dram_tensor("t2_part", [ROWS_PER_CORE, D2], F32)
    t2_full = nc.dram_tensor("t2_full", [DEV_ROWS, D2], F32, addr_space="Shared")

    SELW = 8  # sel slots per DVE instruction

    with tile.TileContext(nc) as tc:
        with (
            tc.tile_pool(name="const", bufs=1) as cp,
            tc.tile_pool(name="sbuf", bufs=2) as sb,
            tc.tile_pool(name="gat", bufs=3) as gp,
            tc.tile_pool(name="selp", bufs=3) as selp,
            tc.tile_pool(name="psum", bufs=2, space="PSUM") as pp,
        ):
            # resident constants
            w1_t = cp.tile([D1, D1], F32)
            nc.sync.dma_start(w1_t[:], W1[:])
            w1b_t = cp.tile([D1, D1], BF16)
            nc.sync.dma_start(w1b_t[:], W1b[:])
            w2b_t = cp.tile([D1, D2], BF16)
            nc.sync.dma_start(w2b_t[:], W2b[:])
            b1_t = cp.tile([1, D1], F32)
            nc.sync.dma_start(b1_t[:], b1r[:])
            b2_t = cp.tile([1, D2], F32)
            nc.sync.dma_start(b2_t[:], b2r[:])
            iota_t = cp.tile([128, 128], F32)
            nc.sync.dma_start(iota_t[:], iota[:])
            idb_t = cp.tile([128, 128], BF16)
            nc.sync.dma_start(idb_t[:], identb[:])
            idf_t = cp.tile([128, 128], F32)
            nc.sync.dma_start(idf_t[:], identf[:])
            dstloc_t = cp.tile([128, NSLOT], F32)
            nc.sync.dma_start(dstloc_t[:], dstloc_d[:])
            dinv_t = cp.tile([128, NBLK], F32)
            nc.sync.dma_start(dinv_t[:], dinv_d[:])
            sqd_t = cp.tile([1, ROWS_PER_CORE], F32)
            nc.sync.dma_start(sqd_t[:], sqd_d[:])

            # ---------------- stage A: x1_part = dinv * (x @ W1) -----------
            for b in range(NBLK):
                xb = sb.tile([128, 128], F32, tag="xTb")
                nc.sync.dma_start(xb[:], xT[:, b * 128:(b + 1) * 128])
                ps = pp.tile([128, D1], F32, tag="agg")
                nc.tensor.matmul(ps[:], lhsT=xb[:],
                                 rhs=w1_t[:], start=True, stop=True)
                xa = sb.tile([128, D1], BF16, tag="xa")
                nc.scalar.activation(xa[:], ps[:],
                                     mybir.ActivationFunctionType.Copy,
                                     scale=dinv_t[:, b:b + 1])
                nc.sync.dma_start(x1_part[b * 128:(b + 1) * 128, :], xa[:])

            nc.gpsimd.collective_compute(
                "AllGather", mybir.AluOpType.bypass,
                replica_groups=[list(range(NC))],
                ins=[x1_part[:]], outs=[x1_full[:]],
            )

            # ---------------- generic aggregation layer --------------------
            def agg_layer(tblA, tblB, d_out, sel_dt, b_row, ident_tile,
                          self_tbl, epilogue):
                """For each block: psum = sqd x b_row  +  I @ self_tbl[block]
                + sum_chunks sel^T @ gathered; epilogue(b, psum)."""
                gathers = []  # (is_a, gi) in emission order
                ia = ib = 0
                while ia < nIA or ib < nIB:
                    # interleave by fraction of stream completed
                    if ib >= nIB or (ia < nIA and ia * nIB <= ib * nIA):
                        gathers.append((True, ia)); ia += 1
                    else:
                        gathers.append((False, ib)); ib += 1

                gtiles = {}
                for is_a, gi in gathers:
                    it = sb.tile([128, NWIRE], I16, tag="idx")
                    nc.sync.dma_start(it[:], (idxA_d if is_a else idxB_d)[gi])
                    g = gp.tile([128, CHUNKS_PER_GATHER, d_out], sel_dt,
                                tag="gA" if is_a else "gB")
                    nc.gpsimd.dma_gather(
                        out_ap=g[:], in_ap=(tblA if is_a else tblB),
                        idxs_ap=it[:], num_idxs=CHUNKS_PER_GATHER * 128,
                        num_idxs_reg=CHUNKS_PER_GATHER * 128,
                        elem_size=d_out, single_packet=False,
                    )
                    gtiles[(is_a, gi)] = g

                # selection matrices, SELW slots at a time
                stiles = {}
                for j0 in range(0, NSLOT, SELW):
                    w = min(SELW, NSLOT - j0)
                    st = selp.tile([128, SELW * 128], sel_dt, tag="sel")
                    base = dstloc_t[:, j0:j0 + w]
                    in0 = AP(dstloc_t.tensor, base.offset,
                             [base.ap[0], [1, w], [0, 128]])
                    it0 = iota_t[:]
                    in1 = AP(iota_t.tensor, it0.offset,
                             [it0.ap[0], [0, w], [1, 128]])
                    nc.vector.tensor_tensor(out=st[:, :w * 128], in0=in0,
                                            in1=in1,
                                            op=mybir.AluOpType.is_equal)
                    stiles[j0] = st

                for b in range(NBLK):
                    ps = pp.tile([128, d_out], F32,
                                 tag="agg" if d_out == D1 else "agg2")
                    nc.tensor.matmul(ps[:], lhsT=sqd_t[:, b * 128:(b + 1) * 128],
                                     rhs=b_row[:], start=True, stop=False)
                    slf = sb.tile([128, d_out], sel_dt, tag="slf")
                    nc.sync.dma_start(slf[:], self_tbl[b * 128:(b + 1) * 128, :])
                    nc.tensor.matmul(ps[:], lhsT=ident_tile[:], rhs=slf[:],
                                     start=False, stop=False)
                    for j in range(CC):
                        is_a = j < kA
                        k = kA if is_a else kB
                        sg = b * k + (j if is_a else j - kA)
                        gi, pos = sg // CHUNKS_PER_GATHER, sg % CHUNKS_PER_GATHER
                        g = gtiles[(is_a, gi)]
                        jj = b * CC + j
                        st = stiles[(jj // SELW) * SELW]
                        off = (jj % SELW) * 128
                        nc.tensor.matmul(
                            ps[:], lhsT=st[:, off:off + 128],
                            rhs=g[:, pos, :],
                            start=False, stop=(j == CC - 1))
                    epilogue(b, ps)

            # ---------------- layer 1 --------------------------------------
            def l1_epilogue(b, ps):
                h = sb.tile([128, D1], BF16, tag="h")
                nc.scalar.activation(h[:], ps[:],
                                     mybir.ActivationFunctionType.Relu,
                                     scale=dinv_t[:, b:b + 1])
                pt = pp.tile([128, 128], BF16, tag="tp")
                nc.tensor.transpose(pt[:], h[:], idb_t[:])
                ht = sb.tile([128, 128], BF16, tag="ht")
                nc.vector.tensor_copy(ht[:], pt[:])
                p2 = pp.tile([128, D2], F32, tag="agg2")
                nc.tensor.matmul(p2[:], lhsT=ht[:], rhs=w2b_t[:],
                                 start=True, stop=True)
                t2 = sb.tile([128, D2], F32, tag="t2")
                nc.scalar.activation(t2[:], p2[:],
                                     mybir.ActivationFunctionType.Copy,
                                     scale=dinv_t[:, b:b + 1])
                nc.sync.dma_start(t2_part[b * 128:(b + 1) * 128, :], t2[:])

            agg_layer(x1_full[0:SPLIT, :], x1_full[SPLIT:DEV_ROWS, :], D1,
                      BF16, b1_t, idb_t, x1_part, l1_epilogue)

            nc.gpsimd.collective_compute(
                "AllGather", mybir.AluOpType.bypass,
                replica_groups=[list(range(NC))],
                ins=[t2_part[:]], outs=[t2_full[:]],
            )

            # ---------------- layer 2 --------------------------------------
            def l2_epilogue(b, ps):
                z = sb.tile([128, D2], F32, tag="z")
                nc.scalar.activation(z[:], ps[:],
                                     mybir.ActivationFunctionType.Copy,
                                     scale=dinv_t[:, b:b + 1])
                nc.sync.dma_start(zcat[b * 128:(b + 1) * 128, :], z[:])

            agg_layer(t2_full[0:SPLIT, :], t2_full[SPLIT:DEV_ROWS, :], D2,
                      F32, b2_t, idf_t, t2_part, l2_epilogue)

    nc.compile()
    return nc


# ----------------------------------------------------------------------------

def kernel(x, edge_index, y_edge_index, W1, b1, W_mu, b_mu, W_ls, b_ls):
    global LAST_RESULTS
    try:  # enable NTFF profiling under axon when available (no-op otherwise)
        from trn_agent_boot.trn_boot import _ntff_profile_via_ctypes
        from antenv.axon_hooks import set_axon_ntff_profile_hook, \
            get_axon_ntff_profile_hook
        if get_axon_ntff_profile_hook() is None:
            set_axon_ntff_profile_hook(
                _ntff_profile_via_ctypes('/opt/axon/libaxon_pjrt.so'))
    except Exception:
        pass

    x = np.asarray(x, np.float32)
    W1 = np.asarray(W1, np.float32)
    b1 = np.asarray(b1, np.float32)
    W2 = np.concatenate([np.asarray(W_mu, np.float32),
                         np.asarray(W_ls, np.float32)], axis=1)
    b2 = np.concatenate([np.asarray(b_mu, np.float32),
                         np.asarray(b_ls, np.float32)])

    per_core, kA, kB, devrow_node, node_devrow = _preprocess(
        edge_index, y_edge_index)

    nc = _build_program(kA, kB)

    iota_np = np.tile(np.arange(128, dtype=np.float32)[None, :], (128, 1))
    ident_np = np.eye(128, dtype=np.float32)

    in_maps = []
    for c in range(NC):
        pc = per_core[c]
        rows = pc["rows"]
        xTc = np.zeros((128, ROWS_PER_CORE), np.float32)
        valid = rows >= 0
        xTc[:, valid] = x[rows[valid]].T
        in_maps.append(dict(
            xT=xTc, W1=W1, W1b=W1.astype(ml_dtypes.bfloat16),
            W2b=W2.astype(ml_dtypes.bfloat16),
            b1r=b1[None, :], b2r=b2[None, :],
            iota=iota_np, identb=ident_np.astype(ml_dtypes.bfloat16),
            identf=ident_np,
            dstloc=pc["dstloc"], dinv_sb=pc["dinv_sb"], sqd_row=pc["sqd_row"],
            idxA=pc["idxA"].reshape(kA, 128, -1),
            idxB=pc["idxB"].reshape(kB, 128, -1),
        ))

    res = run_bass_kernel_spmd(nc, in_maps, list(range(NC)))
    LAST_RESULTS = res

    z_dev = np.concatenate([res.results[c]["zcat"] for c in range(NC)], axis=0)
    z = z_dev[node_devrow]  # [N, 64]
    return z[:, :32].astype(np.float32), z[:, 32:].astype(np.float32)


# revision 13
# speedup vs baseline: 1.6909x; 1.6909x over previous
"""GCN encoder (2-layer VGAE-style) on 8 Trainium2 NeuronCores.

Strategy (graph/data parallel, per sharding hint):
- Destination nodes are partitioned across the 8 cores (6250 each); the small
  weight matrices are replicated.
- Each core aggregates messages for its own destination nodes.  Message
  gathers use dma_gather (int16 indices -> the feature table is addressed in
  two slices split at device-row 32000).
- Host-side graph preprocessing ("METIS-like" partitioning per the hint):
  per-core nodes are re-packed into 50 blocks of <=128 nodes balancing the
  per-block edge counts so every core runs the identical SPMD program;
  normalization constants (deg/dinv, standard cached gcn_norm metadata) are
  computed from edge_index on the host.
- Layer-1 table x1 = dinv * (x @ W1) is built shard-wise on device and
  AllGathered (bf16).  Layer-2 table t2 = dinv * (h @ [W_mu|W_ls]) likewise
  (f32).  Self-loop terms enter via an identity matmul; biases via a rank-1
  (sqrt(deg) x b) PSUM-init matmul, so the whole normalization happens on
  device.
"""

import os
import sys

sys.path.insert(0, "/opt/trn_rl_repo")

import numpy as np
import ml_dtypes

import concourse.bass as bass
import concourse.bacc as bacc
import concourse.mybir as mybir
import concourse.tile as tile
from concourse.bass import AP
from concourse.bass_utils import run_bass_kernel_spmd

# ----------------------------------------------------------------------------
N = 50000
NC = 8
NBPC = N // NC            # 6250 nodes per core
NBLK = 50                 # psum blocks per core
ROWS_PER_CORE = NBLK * 128    # 6400 device rows per core
DEV_ROWS = NC * ROWS_PER_CORE  # 51200
SPLIT = 5 * ROWS_PER_CORE      # 32000: table A = dev rows [0, 32000)
D1 = 128                  # input / hidden feature dim
D2 = 64                   # concat(mu, logstd) output dim
CHUNKS_PER_GATHER = 32    # 5120 indices per dma_gather instruction

F32 = mybir.dt.float32
BF16 = mybir.dt.bfloat16
I16 = mybir.dt.int16

LAST_RESULTS = None       # test harness reads profiling info from here


# ----------------------------------------------------------------------------
# Host-side graph preprocessing
# ----------------------------------------------------------------------------

def _pack_core(nodes, degA, degB):
    """Pack `nodes` into NBLK blocks of <=128 nodes, balancing A/B edge loads.
    Returns (blocks: list[list[node]], maxA, maxB)."""
    order = np.argsort(-(degA + degB), kind="stable")
    loadA = np.zeros(NBLK, np.int64)
    loadB = np.zeros(NBLK, np.int64)
    cnt = np.zeros(NBLK, np.int64)
    blocks = [[] for _ in range(NBLK)]
    wA = 1.0 / max(1.0, degA.sum() / NBLK)   # normalize per-stream loads
    wB = 1.0 / max(1.0, degB.sum() / NBLK)
    for idx in order:
        da, db = degA[idx], degB[idx]
        # feasible blocks (node count)
        score = np.maximum((loadA + da) * wA, (loadB + db) * wB)
        score[cnt >= 128] = np.inf
        b = int(np.argmin(score))
        blocks[b].append(nodes[idx])
        loadA[b] += da
        loadB[b] += db
        cnt[b] += 1
    return blocks, int(loadA.max()), int(loadB.max())


def _preprocess(edge_index, y_edge_index):
    ei = np.concatenate([np.asarray(edge_index), np.asarray(y_edge_index)], axis=1)
    src = ei[0].astype(np.int64)
    dst = ei[1].astype(np.int64)

    deg = np.bincount(dst, minlength=N).astype(np.float64) + 1.0
    dinv = (1.0 / np.sqrt(deg)).astype(np.float32)
    sqd = np.sqrt(deg).astype(np.float32)

    is_a_src = src < SPLIT // ROWS_PER_CORE * NBPC  # src node in cores 0-4

    # degrees per node split by src-table half (excluding self loops, which we
    # handle via the identity matmul; existing (i,i) edges stay in the list)
    degA_all = np.bincount(dst[is_a_src], minlength=N)
    degB_all = np.bincount(dst[~is_a_src], minlength=N)

    core_blocks = []
    maxA = maxB = 0
    for c in range(NC):
        lo, hi = c * NBPC, (c + 1) * NBPC
        nodes = np.arange(lo, hi)
        blocks, mA, mB = _pack_core(nodes, degA_all[lo:hi], degB_all[lo:hi])
        core_blocks.append(blocks)
        maxA = max(maxA, mA)
        maxB = max(maxB, mB)

    kA = max(1, -(-maxA // 128))
    kB = max(1, -(-maxB // 128))
    assert NBLK * kA % 1 == 0

    # device row assignment
    node_devrow = np.empty(N, np.int64)
    devrow_node = np.full(DEV_ROWS, -1, np.int64)
    for c in range(NC):
        for b, blk in enumerate(core_blocks[c]):
            base = c * ROWS_PER_CORE + b * 128
            for p, n in enumerate(blk):
                node_devrow[n] = base + p
                devrow_node[base + p] = n
    assert (node_devrow >= 0).all()
    # consistency: src < 31250  <=>  devrow < SPLIT
    assert ((node_devrow < SPLIT) == (np.arange(N) < 5 * NBPC)).all()

    src_dev = node_devrow[src]
    dst_dev = node_devrow[dst]

    nIA = -(-NBLK * kA // CHUNKS_PER_GATHER)
    nIB = -(-NBLK * kB // CHUNKS_PER_GATHER)

    per_core = []
    for c in range(NC):
        m = (dst >= c * NBPC) & (dst < (c + 1) * NBPC)
        es, ed = src_dev[m], dst_dev[m] - c * ROWS_PER_CORE
        blk = ed // 128
        dloc = ed % 128
        isa = es < SPLIT

        # slot-major edge layout: for each block, A-edges then B-edges
        idxA = np.zeros((nIA, CHUNKS_PER_GATHER, 128), np.int16)
        idxB = np.zeros((nIB, CHUNKS_PER_GATHER, 128), np.int16)
        nslot = NBLK * (kA + kB)
        dstloc = np.full((128, nslot), -1.0, np.float32)

        for b in range(NBLK):
            for is_a in (True, False):
                sel = (blk == b) & (isa == is_a)
                e_src = es[sel] if is_a else es[sel] - SPLIT
                e_dl = dloc[sel]
                o = np.argsort(e_src, kind="stable")  # HBM row locality
                e_src, e_dl = e_src[o], e_dl[o]
                k = kA if is_a else kB
                assert len(e_src) <= k * 128, (c, b, is_a, len(e_src))
                for t in range(len(e_src)):
                    j, lane = t // 128, t % 128
                    slot_g = b * k + j          # slot in the A/B stream
                    gi, pos = slot_g // CHUNKS_PER_GATHER, slot_g % CHUNKS_PER_GATHER
                    if is_a:
                        idxA[gi, pos, lane] = e_src[t]
                    else:
                        idxB[gi, pos, lane] = e_src[t]
                    jj = b * (kA + kB) + (j if is_a else kA + j)
                    dstloc[lane, jj] = e_dl[t]

        # wire layout: [nI, 128, 400] where [g, i%16 + 16r, i//16] = idx_i
        def wire(a):
            nI = a.shape[0]
            flat = a.reshape(nI, CHUNKS_PER_GATHER * 128)
            w = flat.reshape(nI, -1, 16).transpose(0, 2, 1)  # [nI, 16, n/16]
            return np.tile(w, (1, 8, 1)).astype(np.int16)

        rows = devrow_node[c * ROWS_PER_CORE:(c + 1) * ROWS_PER_CORE]
        valid = rows >= 0
        dinv_sb = np.zeros((128, NBLK), np.float32)
        sqd_row = np.zeros((1, ROWS_PER_CORE), np.float32)
        dv = np.zeros(ROWS_PER_CORE, np.float32)
        dv[valid] = dinv[rows[valid]]
        dinv_sb[:, :] = dv.reshape(NBLK, 128).T
        sq = np.zeros(ROWS_PER_CORE, np.float32)
        sq[valid] = sqd[rows[valid]]
        sqd_row[0] = sq

        per_core.append(dict(
            idxA=wire(idxA), idxB=wire(idxB), dstloc=dstloc,
            dinv_sb=dinv_sb, sqd_row=sqd_row, rows=rows,
        ))

    return per_core, kA, kB, devrow_node, node_devrow


# ----------------------------------------------------------------------------
# Device program
# ----------------------------------------------------------------------------

def _build_program(kA, kB):
    nIA = -(-NBLK * kA // CHUNKS_PER_GATHER)
    nIB = -(-NBLK * kB // CHUNKS_PER_GATHER)
    CC = kA + kB
    NSLOT = NBLK * CC
    NWIRE = CHUNKS_PER_GATHER * 128 // 16  # 400

    nc = bacc.Bacc("TRN2", target_bir_lowering=False, debug=False,
                   num_devices=NC, num_swdge_queues=4)

    # inputs
    xT = nc.dram_tensor("xT", [128, ROWS_PER_CORE], F32, kind="ExternalInput")
    W1 = nc.dram_tensor("W1", [D1, D1], F32, kind="ExternalInput")
    W1b = nc.dram_tensor("W1b", [D1, D1], BF16, kind="ExternalInput")
    W2b = nc.dram_tensor("W2b", [D1, D2], BF16, kind="ExternalInput")
    b1r = nc.dram_tensor("b1r", [1, D1], F32, kind="ExternalInput")
    b2r = nc.dram_tensor("b2r", [1, D2], F32, kind="ExternalInput")
    iota = nc.dram_tensor("iota", [128, 128], F32, kind="ExternalInput")
    identb = nc.dram_tensor("identb", [128, 128], BF16, kind="ExternalInput")
    identf = nc.dram_tensor("identf", [128, 128], F32, kind="ExternalInput")
    dstloc_d = nc.dram_tensor("dstloc", [128, NSLOT], F32, kind="ExternalInput")
    dinv_d = nc.dram_tensor("dinv_sb", [128, NBLK], F32, kind="ExternalInput")
    sqd_d = nc.dram_tensor("sqd_row", [1, ROWS_PER_CORE], F32, kind="ExternalInput")
    idxA_d = nc.dram_tensor("idxA", [nIA, 128, NWIRE], I16, kind="ExternalInput")
    idxB_d = nc.dram_tensor("idxB", [nIB, 128, NWIRE], I16, kind="ExternalInput")

    zcat = nc.dram_tensor("zcat", [ROWS_PER_CORE, D2], F32, kind="ExternalOutput")

    # internal DRAM
    x1_part = nc.dram_tensor("x1_part", [ROWS_PER_CORE, D1], BF16)
    x1_full = nc.dram_tensor("x1_full", [DEV_ROWS, D1], BF16, addr_space="Shared")
    t2_part = nc.dram_tensor("t2_part", [ROWS_PER_CORE, D2], F32)
    t2_full = nc.dram_tensor("t2_full", [DEV_ROWS, D2], F32, addr_space="Shared")

    SELW = 16  # sel slots per DVE instruction

    with tile.TileContext(nc) as tc:
        with (
            tc.tile_pool(name="const", bufs=1) as cp,
            tc.tile_pool(name="sbuf", bufs=2) as sb,
            tc.tile_pool(name="gat", bufs=8) as gp,
            tc.tile_pool(name="selp", bufs=2) as selp,
            tc.tile_pool(name="psum", bufs=2, space="PSUM") as pp,
            tc.tile_pool(name="psum3", bufs=3, space="PSUM") as pp3,
        ):
            # resident constants
            w1_t = cp.tile([D1, D1], F32)
            nc.sync.dma_start(w1_t[:], W1[:])
            w1b_t = cp.tile([D1, D1], BF16)
            nc.sync.dma_start(w1b_t[:], W1b[:])
            w2b_t = cp.tile([D1, D2], BF16)
            nc.sync.dma_start(w2b_t[:], W2b[:])
            b1_t = cp.tile([1, D1], F32)
            nc.sync.dma_start(b1_t[:], b1r[:])
            b2_t = cp.tile([1, D2], F32)
            nc.sync.dma_start(b2_t[:], b2r[:])
            iota_t = cp.tile([128, 128], F32)
            nc.sync.dma_start(iota_t[:], iota[:])
            idb_t = cp.tile([128, 128], BF16)
            nc.sync.dma_start(idb_t[:], identb[:])
            idf_t = cp.tile([128, 128], F32)
            nc.sync.dma_start(idf_t[:], identf[:])
            dstloc_t = cp.tile([128, NSLOT], F32)
            nc.sync.dma_start(dstloc_t[:], dstloc_d[:])
            dinv_t = cp.tile([128, NBLK], F32)
            nc.sync.dma_start(dinv_t[:], dinv_d[:])
            sqd_t = cp.tile([1, ROWS_PER_CORE], F32)
            nc.sync.dma_start(sqd_t[:], sqd_d[:])

            # ---------------- stage A: x1_part = dinv * (x @ W1) -----------
            for b in range(NBLK):
                if b % 4 == 0:
                    xb4 = sb.tile([128, 512], F32, tag="xTb")
                    w = min(512, (NBLK - b) * 128)
                    nc.sync.dma_start(xb4[:, :w], xT[:, b * 128:b * 128 + w])
                ps = pp3.tile([128, D1], F32, tag="agg")
                nc.tensor.matmul(ps[:], lhsT=xb4[:, (b % 4) * 128:(b % 4 + 1) * 128],
                                 rhs=w1_t[:], start=True, stop=True)
                xa = sb.tile([128, D1], BF16, tag="xa")
                nc.scalar.activation(xa[:], ps[:],
                                     mybir.ActivationFunctionType.Copy,
                                     scale=dinv_t[:, b:b + 1])
                nc.sync.dma_start(x1_part[b * 128:(b + 1) * 128, :], xa[:])

            nc.gpsimd.collective_compute(
                "AllGather", mybir.AluOpType.bypass,
                replica_groups=[list(range(NC))],
                ins=[x1_part[:]], outs=[x1_full[:]],
            )

            # ---------------- generic aggregation layer --------------------
            def agg_layer(tblA, tblB, d_out, sel_dt, b_row, ident_tile,
                          self_tbl, epilogue):
                """For each block: psum = sqd x b_row  +  I @ self_tbl[block]
                + sum_chunks sel^T @ gathered; epilogue(b, psum)."""
                gathers = []  # (is_a, gi) in emission order
                ia = ib = 0
                while ia < nIA or ib < nIB:
                    # interleave by fraction of stream completed
                    if ib >= nIB or (ia < nIA and ia * nIB <= ib * nIA):
                        gathers.append((True, ia)); ia += 1
                    else:
                        gathers.append((False, ib)); ib += 1

                gtiles = {}
                for is_a, gi in gathers:
                    it = gp.tile([128, NWIRE], I16, tag="idx")
                    nc.sync.dma_start(it[:], (idxA_d if is_a else idxB_d)[gi])
                    g = gp.tile([128, CHUNKS_PER_GATHER, d_out], sel_dt,
                                tag="gA" if is_a else "gB")
                    nc.gpsimd.dma_gather(
                        out_ap=g[:], in_ap=(tblA if is_a else tblB),
                        idxs_ap=it[:], num_idxs=CHUNKS_PER_GATHER * 128,
                        num_idxs_reg=CHUNKS_PER_GATHER * 128,
                        elem_size=d_out, single_packet=False,
                        queue_num=(0 if (gi % 2 == 0) else 2) if is_a else (1 if (gi % 2 == 0) else 3),
                    )
                    gtiles[(is_a, gi)] = g

                # selection matrices, SELW slots at a time
                stiles = {}
                for j0 in range(0, NSLOT, SELW):
                    w = min(SELW, NSLOT - j0)
                    st = selp.tile([128, SELW * 128], sel_dt, tag="sel")
                    base = dstloc_t[:, j0:j0 + w]
                    in0 = AP(dstloc_t.tensor, base.offset,
                             [base.ap[0], [1, w], [0, 128]])
                    it0 = iota_t[:]
                    in1 = AP(iota_t.tensor, it0.offset,
                             [it0.ap[0], [0, w], [1, 128]])
                    nc.vector.tensor_tensor(out=st[:, :w * 128], in0=in0,
                                            in1=in1,
                                            op=mybir.AluOpType.is_equal)
                    stiles[j0] = st

                for b in range(NBLK):
                    pool_ = pp3 if d_out == D1 else pp
                    ps = pool_.tile([128, d_out], F32,
                                    tag="agg" if d_out == D1 else "agg2")
                    nc.tensor.matmul(ps[:], lhsT=sqd_t[:, b * 128:(b + 1) * 128],
                                     rhs=b_row[:], start=True, stop=False)
                    slf = sb.tile([128, d_out], sel_dt, tag="slf")
                    nc.sync.dma_start(slf[:], self_tbl[b * 128:(b + 1) * 128, :])
                    nc.tensor.matmul(ps[:], lhsT=ident_tile[:], rhs=slf[:],
                                     start=False, stop=False)
                    for j in range(CC):
                        is_a = j < kA
                        k = kA if is_a else kB
                        sg = b * k + (j if is_a else j - kA)
                        gi, pos = sg // CHUNKS_PER_GATHER, sg % CHUNKS_PER_GATHER
                        g = gtiles[(is_a, gi)]
                        jj = b * CC + j
                        st = stiles[(jj // SELW) * SELW]
                        off = (jj % SELW) * 128
                        nc.tensor.matmul(
                            ps[:], lhsT=st[:, off:off + 128],
                            rhs=g[:, pos, :],
                            start=False, stop=(j == CC - 1))
                    epilogue(b, ps)

            # ---------------- layer 1 --------------------------------------
            def l1_epilogue(b, ps):
                h = sb.tile([128, D1], BF16, tag="h")
                nc.scalar.activation(h[:], ps[:],
                                     mybir.ActivationFunctionType.Relu,
                                     scale=dinv_t[:, b:b + 1])
                pt = pp.tile([128, 128], BF16, tag="tp")
                nc.tensor.transpose(pt[:], h[:], idb_t[:])
                ht = sb.tile([128, 128], BF16, tag="ht")
                nc.vector.tensor_copy(ht[:], pt[:])
                p2 = pp.tile([128, D2], F32, tag="agg2")
                nc.tensor.matmul(p2[:], lhsT=ht[:], rhs=w2b_t[:],
                                 start=True, stop=True)
                t2 = sb.tile([128, D2], F32, tag="t2")
                nc.scalar.activation(t2[:], p2[:],
                                     mybir.ActivationFunctionType.Copy,
                                     scale=dinv_t[:, b:b + 1])
                nc.sync.dma_start(t2_part[b * 128:(b + 1) * 128, :], t2[:])

            agg_layer(x1_full[0:SPLIT, :], x1_full[SPLIT:DEV_ROWS, :], D1,
                      BF16, b1_t, idb_t, x1_part, l1_epilogue)

            nc.gpsimd.collective_compute(
                "AllGather", mybir.AluOpType.bypass,
                replica_groups=[list(range(NC))],
                ins=[t2_part[:]], outs=[t2_full[:]],
            )

            # ---------------- layer 2 --------------------------------------
            def l2_epilogue(b, ps):
                z = sb.tile([128, D2], F32, tag="z")
                nc.scalar.activation(z[:], ps[:],
                                     mybir.ActivationFunctionType.Copy,
                                     scale=dinv_t[:, b:b + 1])
                nc.sync.dma_start(zcat[b * 128:(b + 1) * 128, :], z[:])

            agg_layer(t2_full[0:SPLIT, :], t2_full[SPLIT:DEV_ROWS, :], D2,
                      F32, b2_t, idf_t, t2_part, l2_epilogue)

    nc.compile()
    return nc


# ----------------------------------------------------------------------------

def kernel(x, edge_index, y_edge_index, W1, b1, W_mu, b_mu, W_ls, b_ls):
    global LAST_RESULTS
    try:  # enable NTFF profiling under axon when available (no-op otherwise)
        from trn_agent_boot.trn_boot import _ntff_profile_via_ctypes
        from antenv.axon_hooks import set_axon_ntff_profile_hook, \
            get_axon_ntff_profile_hook
        if get_axon_ntff_profile_hook() is None:
            set_axon_ntff_profile_hook(
                _ntff_profile_via_ctypes('/opt/axon/libaxon_pjrt.so'))
    except Exception:
        pass

    x = np.asarray(x, np.float32)
    W1 = np.asarray(W1, np.float32)
    b1 = np.asarray(b1, np.float32)
    W2 = np.concatenate([np.asarray(W_mu, np.float32),
                         np.asarray(W_ls, np.float32)], axis=1)
    b2 = np.concatenate([np.asarray(b_mu, np.float32),
                         np.asarray(b_ls, np.float32)])

    per_core, kA, kB, devrow_node, node_devrow = _preprocess(
        edge_index, y_edge_index)

    nc = _build_program(kA, kB)

    iota_np = np.tile(np.arange(128, dtype=np.float32)[None, :], (128, 1))
    ident_np = np.eye(128, dtype=np.float32)

    in_maps = []
    for c in range(NC):
        pc = per_core[c]
        rows = pc["rows"]
        xTc = np.zeros((128, ROWS_PER_CORE), np.float32)
        valid = rows >= 0
        xTc[:, valid] = x[rows[valid]].T
        in_maps.append(dict(
            xT=xTc, W1=W1, W1b=W1.astype(ml_dtypes.bfloat16),
            W2b=W2.astype(ml_dtypes.bfloat16),
            b1r=b1[None, :], b2r=b2[None, :],
            iota=iota_np, identb=ident_np.astype(ml_dtypes.bfloat16),
            identf=ident_np,
            dstloc=pc["dstloc"], dinv_sb=pc["dinv_sb"], sqd_row=pc["sqd_row"],
            idxA=pc["idxA"], idxB=pc["idxB"],
        ))

    res = run_bass_kernel_spmd(nc, in_maps, list(range(NC)))
    LAST_RESULTS = res

    z_dev = np.concatenate([res.results[c]["zcat"] for c in range(NC)], axis=0)
    z = z_dev[node_devrow]  # [N, 64]
    return z[:, :32].astype(np.float32), z[:, 32:].astype(np.float32)
